# revision 19
# baseline (speedup 1.0000x reference)
"""Trainium2 Bass kernel for nn_BaichuanAttention_4801773437527 (v3).

Sequence-sharded across 8 NeuronCores: core c handles 512 query rows
(batch c//4, seq block (c%4)*512). No collectives.

v3 changes vs v2:
 - Q and K projections in fp8-e4m3 with DoubleRow perf mode (2 packed
   contraction tiles per matmul): ~1.5-2x PE rate on 330us of matmuls.
   Inputs are scaled x64 on host (fp8 mid-range); the resulting x4096
   scale on q and k rides linearly through smoothing/rope and is divided
   out inside the softmax exp's scale argument -- zero extra ops.
   Numerically safe: q/k only feed scores, which are tiny (probs shift
   measured 5e-6 on CPU).
 - V projection stays bf16 (v feeds the output directly).
 - paired-accumulator kv matmuls: one LDWEIGHTS serves prev+own halves
 - o_proj weight pool opened early; first tile prefetched mid-attention
 - first weight tile DMA issued before the hidden-state DMAs
"""
import sys
sys.path.insert(0, '/opt/trn_rl_repo')
from contextlib import ExitStack
import numpy as np

B, S, HID = 2, 2048, 4096
H, KV, D = 32, 8, 128
WINDOW = 512
CHUNK = 512
NCORES = 8
ROPE_THETA = 10000.0
KT = HID // 128               # 32 contraction tiles (bf16); 16 fp8 pairs
SCALE = float(D) ** -0.5
FP8_SCALE = 64.0              # per-operand fp8 input scale
SCALE_QK = SCALE / (FP8_SCALE ** 4)   # q,k each carry x(64*64)

QRANGE = [(0, 128), (0, 256), (0, 384), (0, 512),
          (0, 512), (128, 384), (256, 256), (384, 128)]
KT_ORDER = [3, 0, 1, 2, 4, 5, 6, 7]

_PROGRAM = None
TRACE = False
_LAST_RESULTS = None


def _apply_patches():
    """This walrus build allows 1 sync wait per instruction (2 for
    EventSemaphore). Spill extra waits onto same-engine no-ops."""
    import concourse.mybir as mybir
    import concourse.tile as tile
    from concourse.vector_clock import ScopedClock

    if getattr(tile.TileContext, "_wait_patch_applied", False):
        return

    orig_lower = tile.TileContext._lower_ordered_insts
    counter = [0]

    def spill(ordered):
        for insts in ordered.values():
            new_insts = []
            for inst in insts:
                si = getattr(inst, "sync_info", None)
                if si is not None and type(inst).__name__.startswith("Inst"):
                    waits = list(si.on_wait)
                    cap = 2 if isinstance(inst, mybir.InstEventSemaphore) else 1
                    if len(waits) > cap:
                        for w in waits[cap:]:
                            counter[0] += 1
                            new_insts.append(mybir.InstNoOp(
                                name=f"wspill-{counter[0]}",
                                sync_info=mybir.SyncInfo(on_wait=[w], on_update=[]),
                                bass_nofuse=True,
                                engine=inst.engine,
                            ))
                        inst.sync_info = mybir.SyncInfo(
                            on_wait=waits[:cap], on_update=list(si.on_update))
                new_insts.append(inst)
            insts[:] = new_insts

    def patched_lower(self, ordered):
        spill(ordered)
        return orig_lower(self, ordered)

    def patched_drain_and_barrier(self, tick_clock, wait_clock):
        nc = self.nc
        collector = nc.sync.nop(nofuse=True)
        wait_clock.add_sem_waits(
            collector.ins, ScopedClock({None: tick_clock.global_clock}))
        si = collector.ins.sync_info
        waits = list(si.on_wait) if si is not None else []
        if len(waits) > 1:
            collector.ins.sync_info = mybir.SyncInfo(
                on_wait=[waits[0]], on_update=list(si.on_update))
            for w in waits[1:]:
                n = nc.sync.nop(nofuse=True)
                n.ins.sync_info = mybir.SyncInfo(on_wait=[w], on_update=[])
        nc.sync.drain()
        nc.all_engine_barrier()
        assert self.sems is not None
        popped = nc._tile_sem_poison_stack.pop()
        assert popped is self._sem_poison
        nc.clear_and_free_semaphores(list(self.sems.allocated().values()))
        nc.all_engine_barrier()

    tile.TileContext._lower_ordered_insts = patched_lower
    tile.TileContext._drain_and_barrier = patched_drain_and_barrier
    tile.TileContext._wait_patch_applied = True


def _build_program():
    import concourse.bass as bass
    import concourse.mybir as mybir
    import concourse.tile as tile
    from concourse.masks import make_identity

    _apply_patches()

    f32 = mybir.dt.float32
    bf16 = mybir.dt.bfloat16
    fp8 = mybir.dt.float8e4
    DR = mybir.MatmulPerfMode.DoubleRow
    MUL = mybir.AluOpType.mult
    ADD = mybir.AluOpType.add
    EXP = mybir.ActivationFunctionType.Exp

    nc = bass.Bass()
    # bf16 hidden (for V); fp8 hidden pre-paired [p, j, r, t] (for Q/K)
    hTprev = nc.dram_tensor("htprev", [128, KT, 512], bf16, kind="ExternalInput")
    hTown = nc.dram_tensor("htown", [128, KT, 512], bf16, kind="ExternalInput")
    h8prev = nc.dram_tensor("h8prev", [128, 16, 2, 512], fp8, kind="ExternalInput")
    h8own = nc.dram_tensor("h8own", [128, 16, 2, 512], fp8, kind="ExternalInput")
    wv = nc.dram_tensor("wv", [128, 8, KT, 128], bf16, kind="ExternalInput")
    wk8 = nc.dram_tensor("wk8", [128, 8, 16, 2, 128], fp8, kind="ExternalInput")
    wq8 = nc.dram_tensor("wq8", [128, 32, 16, 2, 128], fp8, kind="ExternalInput")
    wodr = nc.dram_tensor("wodr", [128, 8, KT, 512], bf16, kind="ExternalInput")
    costab = nc.dram_tensor("costab", [128, 1024], bf16, kind="ExternalInput")
    sintab = nc.dram_tensor("sintab", [128, 1024], bf16, kind="ExternalInput")
    maskst = nc.dram_tensor("maskst", [128, 8, 512], bf16, kind="ExternalInput")
    filt = nc.dram_tensor("filt", [128, 32], f32, kind="ExternalInput")
    rotm = nc.dram_tensor("rotm", [128, 128], bf16, kind="ExternalInput")
    out = nc.dram_tensor("out", [CHUNK, HID], f32, kind="ExternalOutput")
    out_r = out[:].rearrange("(st p) h -> st p h", p=128)   # [4,128,4096]

    with tile.TileContext(nc) as tc, ExitStack() as top:
        constp = top.enter_context(tc.tile_pool(name="const", bufs=1))
        ident_f = constp.tile([128, 128], f32, tag="identf")
        make_identity(nc, ident_f[:])
        ident = constp.tile([128, 128], bf16, tag="ident")
        nc.vector.tensor_copy(ident[:], ident_f[:])
        ones2d = constp.tile([128, 128], bf16, tag="ones2d")
        nc.gpsimd.memset(ones2d[:], 1.0)
        rot_sb = constp.tile([128, 128], bf16, tag="rot")
        nc.gpsimd.dma_start(rot_sb[:], rotm[:])
        cos_sb = constp.tile([128, 1024], bf16, tag="cos")
        sin_sb = constp.tile([128, 1024], bf16, tag="sin")
        nc.gpsimd.dma_start(cos_sb[:], costab[:])
        nc.gpsimd.dma_start(sin_sb[:], sintab[:])
        filt_sb = constp.tile([128, 32], f32, tag="filt")
        nc.gpsimd.dma_start(filt_sb[:], filt[:])

        es_at = ExitStack()
        atp = es_at.enter_context(tc.tile_pool(name="atp", bufs=1))
        attnT = atp.tile([128, H, 512], bf16, tag="attnT")

        es_kv = ExitStack()
        kvfix = es_kv.enter_context(tc.tile_pool(name="kvfix", bufs=1))
        kT_all = kvfix.tile([128, KV, 1024], bf16, tag="kTall")
        vT_all = kvfix.tile([128, KV * 8, 128], bf16, tag="vTall")

        es_h8o = ExitStack()
        h8op = es_h8o.enter_context(tc.tile_pool(name="h8op", bufs=1))
        h8own_sb = h8op.tile([128, 16, 2, 512], fp8, tag="h8own")
        es_hbo = ExitStack()
        hbop = es_hbo.enter_context(tc.tile_pool(name="htownp", bufs=1))
        htown_sb = hbop.tile([128, KT, 512], bf16, tag="htown")
        es_vw = ExitStack()
        vw = es_vw.enter_context(tc.tile_pool(name="vw", bufs=2))
        es_h8p = ExitStack()
        h8pp = es_h8p.enter_context(tc.tile_pool(name="h8pp", bufs=1))
        h8prev_sb = h8pp.tile([128, 16, 2, 512], fp8, tag="h8prev")

        # ---- phase Kk: K projection (fp8 DoubleRow) + smooth + rope ----
        vwts = []
        with tc.tile_pool(name="kw8", bufs=2) as kw8, \
             tc.tile_pool(name="kps", bufs=2, space="PSUM") as kps, \
             tc.tile_pool(name="krot", bufs=1, space="PSUM") as krot, \
             tc.tile_pool(name="smpk", bufs=2) as smpk:
            # DMA order: first K weight tile, then own/prev fp8 hidden in
            # chunks, remaining K weights, then next-phase prefetches
            kwts = []
            for i in range(8):
                wt = kw8.tile([128, 16, 2, 128], fp8, tag="kw")
                nc.sync.dma_start(wt[:], wk8[:, i])
                kwts.append(wt)
                if i == 0:
                    for cc in range(4):
                        nc.sync.dma_start(h8own_sb[:, 4 * cc:4 * cc + 4],
                                          h8own[:, 4 * cc:4 * cc + 4])
                    for cc in range(4):
                        nc.sync.dma_start(h8prev_sb[:, 4 * cc:4 * cc + 4],
                                          h8prev[:, 4 * cc:4 * cc + 4])
            # prefetch phase-V inputs while K computes
            for cc in range(4):
                nc.sync.dma_start(htown_sb[:, 8 * cc:8 * cc + 8, :],
                                  hTown[:, 8 * cc:8 * cc + 8, :])
            wt0 = vw.tile([128, KT, 128], bf16, tag="vw", name="vw0")
            nc.sync.dma_start(wt0[:], wv[:, 0])
            vwts.append(wt0)
            for i in range(8):
                wt = kwts[i]
                ps0 = kps.tile([128, 512], f32, tag="kps0")
                ps1 = kps.tile([128, 512], f32, tag="kps1")
                for j in range(16):
                    nc.tensor.matmul(ps1[:], wt[:, j], h8own_sb[:, j],
                                     start=(j == 0), stop=(j == 15),
                                     perf_mode=DR)
                    nc.tensor.matmul(ps0[:], wt[:, j], h8prev_sb[:, j],
                                     start=(j == 0), stop=(j == 15),
                                     perf_mode=DR)
                raw = smpk.tile([128, 1024], bf16, tag="kvraw")
                nc.scalar.copy(raw[:, 0:512], ps0[:])
                nc.scalar.copy(raw[:, 512:1024], ps1[:])
                f0 = filt_sb[:, i:i + 1]
                f1 = filt_sb[:, 8 + i:8 + i + 1]
                tmp = smpk.tile([128, 1024], bf16, tag="smtmp")
                nc.vector.tensor_scalar_mul(tmp[:], raw[:], f1)
                smo = smpk.tile([128, 1024], bf16, tag="smo")
                nc.vector.tensor_copy(smo[:, 0:1], tmp[:, 0:1])
                nc.vector.scalar_tensor_tensor(
                    smo[:, 1:1024], raw[:, 0:1023], f0, tmp[:, 1:1024], MUL, ADD)
                zps = krot.tile([128, 1024], f32, tag="zk")
                nc.tensor.matmul(zps[:, 0:512], rot_sb[:], smo[:, 0:512],
                                 start=True, stop=True)
                nc.tensor.matmul(zps[:, 512:1024], rot_sb[:], smo[:, 512:1024],
                                 start=True, stop=True)
                zsb = smpk.tile([128, 1024], bf16, tag="zsb")
                nc.scalar.copy(zsb[:], zps[:])
                t1 = smpk.tile([128, 1024], bf16, tag="kt1")
                nc.vector.tensor_tensor(t1[:], smo[:], cos_sb[:], MUL)
                t2 = smpk.tile([128, 1024], bf16, tag="kt2")
                nc.vector.tensor_tensor(t2[:], zsb[:], sin_sb[:], MUL)
                nc.vector.tensor_tensor(kT_all[:, i, :], t1[:], t2[:], ADD)
        es_h8p.close()

        # ---- phase V: V projection (bf16) + smooth + transpose ----
        es_hbp = ExitStack()
        hbpp = es_hbp.enter_context(tc.tile_pool(name="htprevp", bufs=1))
        htprev_sb = hbpp.tile([128, KT, 512], bf16, tag="htprev")
        with tc.tile_pool(name="vps", bufs=2, space="PSUM") as vps, \
             tc.tile_pool(name="vtps", bufs=2, space="PSUM") as vtps, \
             tc.tile_pool(name="smpv", bufs=2) as smpv:
            for cc in range(4):
                nc.sync.dma_start(
                    htprev_sb[:, 8 * cc:8 * cc + 8, :],
                    hTprev[:, 8 * cc:8 * cc + 8, :])
            for i in range(1, 8):
                wt = vw.tile([128, KT, 128], bf16, tag="vw")
                nc.sync.dma_start(wt[:], wv[:, i])
                vwts.append(wt)
            for i in range(8):
                wt = vwts[i]
                ps0 = vps.tile([128, 512], f32, tag="vps0")
                ps1 = vps.tile([128, 512], f32, tag="vps1")
                for kt in range(KT):
                    nc.tensor.matmul(ps1[:], wt[:, kt, :], htown_sb[:, kt, :],
                                     start=(kt == 0), stop=(kt == KT - 1))
                    nc.tensor.matmul(ps0[:], wt[:, kt, :], htprev_sb[:, kt, :],
                                     start=(kt == 0), stop=(kt == KT - 1))
                raw = smpv.tile([128, 1024], bf16, tag="vraw")
                nc.scalar.copy(raw[:, 0:512], ps0[:])
                nc.scalar.copy(raw[:, 512:1024], ps1[:])
                f0 = filt_sb[:, 16 + i:16 + i + 1]
                f1 = filt_sb[:, 24 + i:24 + i + 1]
                tmp = smpv.tile([128, 1024], bf16, tag="vtmp")
                nc.vector.tensor_scalar_mul(tmp[:], raw[:], f1)
                smo = smpv.tile([128, 1024], bf16, tag="vsmo")
                nc.vector.tensor_copy(smo[:, 0:1], tmp[:, 0:1])
                nc.vector.scalar_tensor_tensor(
                    smo[:, 1:1024], raw[:, 0:1023], f0, tmp[:, 1:1024], MUL, ADD)
                for tt in range(8):
                    pv = vtps.tile([128, 128], bf16, tag="vtp")
                    nc.tensor.transpose(
                        pv[:], smo[:, tt * 128:(tt + 1) * 128], ident[:])
                    nc.vector.tensor_copy(vT_all[:, i * 8 + tt, :], pv[:])
        es_hbp.close()
        es_vw.close()
        es_hbo.close()

        # ---- phase QA: per-head q projection (fp8 DR) + rope + attention ----
        es_msk = ExitStack()
        maskp = es_msk.enter_context(tc.tile_pool(name="maskp", bufs=1))
        mask_sb = maskp.tile([128, 8, 512], bf16, tag="mask")
        nc.gpsimd.dma_start(mask_sb[:], maskst[:])
        es_wo0 = ExitStack()
        wop = es_wo0.enter_context(tc.tile_pool(name="wop", bufs=3))
        wodr_h = wodr[:].rearrange("p hc (kh ko) c -> p (hc kh) ko c", kh=2)
        owts = [None] * 3     # first 3 half-KT o_proj weight tiles
        with tc.tile_pool(name="qw", bufs=2) as qw, \
             tc.tile_pool(name="qps", bufs=2, space="PSUM") as qps, \
             tc.tile_pool(name="sps", bufs=2, space="PSUM") as sps, \
             tc.tile_pool(name="sums", bufs=2, space="PSUM") as sums, \
             tc.tile_pool(name="pvps", bufs=2, space="PSUM") as pvps, \
             tc.tile_pool(name="prp", bufs=2) as prp, \
             tc.tile_pool(name="qtmp", bufs=2) as qtmp:
            for h in range(H):
                g = h // (H // KV)
                wt = qw.tile([128, 16, 2, 128], fp8, tag="qwt")
                nc.sync.dma_start(wt[:], wq8[:, h])
                if h in (8, 16, 24):
                    oi = (h - 8) // 8
                    owts[oi] = wop.tile([128, 16, 512], bf16, tag="wo",
                                        name=f"wo{oi}")
                    nc.sync.dma_start(owts[oi][:], wodr_h[:, oi])
                ps = qps.tile([128, 512], f32, tag="qps")
                for j in range(16):
                    nc.tensor.matmul(ps[:], wt[:, j], h8own_sb[:, j],
                                     start=(j == 0), stop=(j == 15),
                                     perf_mode=DR)
                qraw = qtmp.tile([128, 512], bf16, tag="qraw")
                nc.scalar.copy(qraw[:], ps[:])
                zq = sps.tile([128, 512], f32, tag="sc", name="zq")
                nc.tensor.matmul(zq[:], rot_sb[:], qraw[:], start=True, stop=True)
                zqs = qtmp.tile([128, 512], bf16, tag="zqs")
                nc.scalar.copy(zqs[:], zq[:])
                t1 = qtmp.tile([128, 512], bf16, tag="qt1")
                nc.vector.tensor_tensor(t1[:], qraw[:], cos_sb[:, 512:1024], MUL)
                t2 = qtmp.tile([128, 512], bf16, tag="qt2")
                nc.vector.tensor_tensor(t2[:], zqs[:], sin_sb[:, 512:1024], MUL)
                qf = qtmp.tile([128, 512], bf16, tag="qf")
                nc.vector.tensor_tensor(qf[:], t1[:], t2[:], ADD)

                probs = prp.tile([128, 8, 512], bf16, tag="probs")
                for kt in range(8):
                    qo, qn = QRANGE[kt]
                    sc = sps.tile([128, 512], f32, tag="sc")
                    nc.tensor.matmul(
                        sc[:, qo:qo + qn],
                        kT_all[:, g, kt * 128:(kt + 1) * 128],
                        qf[:, qo:qo + qn], start=True, stop=True)
                    nc.scalar.activation(
                        probs[:, kt, qo:qo + qn], sc[:, qo:qo + qn],
                        EXP, scale=SCALE_QK)
                    nc.vector.tensor_tensor(
                        probs[:, kt, qo:qo + qn], probs[:, kt, qo:qo + qn],
                        mask_sb[:, kt, qo:qo + qn], MUL)
                sm_ps = sums.tile([128, 512], f32, tag="sum")
                for j, kt in enumerate(KT_ORDER):
                    qo, qn = QRANGE[kt]
                    nc.tensor.matmul(
                        sm_ps[:, qo:qo + qn], ones2d[:],
                        probs[:, kt, qo:qo + qn],
                        start=(j == 0), stop=(j == 7))
                rec = qtmp.tile([128, 512], f32, tag="rec")
                nc.vector.reciprocal(rec[:], sm_ps[:])
                pv = pvps.tile([128, 512], f32, tag="pv")
                for j, kt in enumerate(KT_ORDER):
                    qo, qn = QRANGE[kt]
                    nc.tensor.matmul(
                        pv[:, qo:qo + qn], vT_all[:, g * 8 + kt, :],
                        probs[:, kt, qo:qo + qn],
                        start=(j == 0), stop=(j == 7))
                nc.vector.tensor_tensor(attnT[:, h, :], pv[:], rec[:], MUL)

        # ---- phase O: o_proj (half-KT weight tiles) ----
        with tc.tile_pool(name="ops", bufs=4, space="PSUM") as ops, \
             tc.tile_pool(name="oev", bufs=3) as oev:
            for hc in range(8):
                wts = []
                for kh in range(2):
                    t = 2 * hc + kh
                    if t < 3:
                        wts.append(owts[t])
                    else:
                        wt = wop.tile([128, 16, 512], bf16, tag="wo")
                        nc.sync.dma_start(wt[:], wodr_h[:, t])
                        wts.append(wt)
                for st in range(4):
                    ps = ops.tile([128, 512], f32, tag="ops")
                    for ft in range(KT):
                        nc.tensor.matmul(
                            ps[:], attnT[:, ft, st * 128:(st + 1) * 128],
                            wts[ft // 16][:, ft % 16, :],
                            start=(ft == 0), stop=(ft == KT - 1))
                    ev = oev.tile([128, 512], f32, tag="oev")
                    nc.scalar.copy(ev[:], ps[:])
                    nc.gpsimd.dma_start(
                        out_r[st, :, hc * 512:(hc + 1) * 512], ev[:])
        es_wo0.close()
        es_msk.close()
        es_h8o.close()
        es_kv.close()
        es_at.close()
    return nc


def _rot_matrix():
    R = np.zeros((128, 128), np.float32)
    for d in range(64):
        R[d + 64, d] = -1.0
        R[d, d + 64] = 1.0
    return R


def _host_tables(positions_b, s0):
    """bf16 cos/sin rope tables [128,1024] and {0,1} bf16 mask [128,8,512]."""
    import ml_dtypes
    if s0 > 0:
        pos_prev = positions_b[s0 - 512:s0].astype(np.float64)
    else:
        pos_prev = np.zeros(512, np.float64)
    pos_own = positions_b[s0:s0 + 512].astype(np.float64)
    tpos = np.concatenate([pos_prev, pos_own])                   # [1024]
    inv = 1.0 / (ROPE_THETA ** (np.arange(64, dtype=np.float64) / 64.0))
    ang = inv[:, None] * tpos[None, :]                           # [64,1024]
    cos = np.cos(ang)
    sin = np.sin(ang)
    costab = np.concatenate([cos, cos], axis=0).astype(ml_dtypes.bfloat16)
    sintab = np.concatenate([sin, sin], axis=0).astype(ml_dtypes.bfloat16)

    t_idx = s0 - 512 + np.arange(1024)
    q_idx = s0 + np.arange(512)
    diff = q_idx[None, :] - t_idx[:, None]                       # [1024,512]
    valid = (diff >= 0) & (diff < WINDOW) & (t_idx[:, None] >= 0)
    maskmul = valid.astype(ml_dtypes.bfloat16)                   # [1024,512]
    maskmul = np.ascontiguousarray(
        maskmul.reshape(8, 128, 512).transpose(1, 0, 2))         # [128,8,512]
    return costab, sintab, maskmul


_WCACHE = {}


def _prep_weights(w_pack, w_o):
    import ml_dtypes
    fp8 = ml_dtypes.float8_e4m3
    key = (w_pack.__array_interface__['data'][0],
           w_o.__array_interface__['data'][0])
    if _WCACHE.get('key') == key:
        return _WCACHE['res']
    wp4 = w_pack.reshape(KT, 128, 48, 128)          # [ko, p, ft, c]
    # V tiles (orig f-tiles 40..47), bf16: wv[p, i, ko, c]
    wv = np.ascontiguousarray(
        wp4[:, :, 40:48, :].transpose(1, 2, 0, 3)).astype(ml_dtypes.bfloat16)
    # K tiles (32..39) fp8 x64, paired ko: wk8[p, i, j, r, c]
    wk = (wp4[:, :, 32:40, :] * FP8_SCALE).astype(fp8)   # [ko, p, i, c]
    wk8 = np.ascontiguousarray(
        wk.reshape(16, 2, 128, 8, 128).transpose(2, 3, 0, 1, 4))
    # Q tiles (0..31) fp8 x64
    wq = (wp4[:, :, 0:32, :] * FP8_SCALE).astype(fp8)
    wq8 = np.ascontiguousarray(
        wq.reshape(16, 2, 128, 32, 128).transpose(2, 3, 0, 1, 4))
    # wodr[p, hc, ko, c] = w_o[ko*128+p, hc*512+c]
    wo4 = w_o.reshape(KT, 128, 8, 512)
    wo = np.ascontiguousarray(
        wo4.transpose(1, 2, 0, 3)).astype(ml_dtypes.bfloat16)
    res = (wv, wk8, wq8, wo)
    _WCACHE.update(key=key, res=res)
    return res


def kernel(**inputs) -> np.ndarray:
    global _PROGRAM, _LAST_RESULTS
    from concourse.bass_utils import run_bass_kernel_spmd
    import ml_dtypes
    fp8 = ml_dtypes.float8_e4m3

    hidden = np.ascontiguousarray(inputs["hidden_states"], dtype=np.float32)
    positions = np.asarray(inputs["positions"], dtype=np.int32)
    w_pack = np.ascontiguousarray(inputs["w_pack"], dtype=np.float32)
    w_o = np.ascontiguousarray(inputs["w_o"], dtype=np.float32)
    conv_k = np.asarray(inputs["conv_k"], dtype=np.float32)
    conv_v = np.asarray(inputs["conv_v"], dtype=np.float32)

    if _PROGRAM is None:
        _PROGRAM = _build_program()
    nc = _PROGRAM

    wv, wk8, wq8, wo = _prep_weights(w_pack, w_o)
    filt_arr = np.concatenate(
        [conv_k[0], conv_k[1], conv_v[0], conv_v[1]]).reshape(1, 32)
    filt_arr = np.ascontiguousarray(
        np.tile(filt_arr, (128, 1)), dtype=np.float32)
    rotm = _rot_matrix().astype(ml_dtypes.bfloat16)

    in_maps = []
    for c in range(NCORES):
        b, s0 = c // 4, (c % 4) * CHUNK
        own = hidden[b, s0:s0 + CHUNK]
        prev = hidden[b, s0 - CHUNK:s0] if s0 > 0 else np.zeros_like(own)
        # hT[p, kt, t] = chunk[t, kt*128+p]
        hTprev = np.ascontiguousarray(
            prev.T.reshape(KT, 128, 512).transpose(1, 0, 2)
        ).astype(ml_dtypes.bfloat16)
        hTown = np.ascontiguousarray(
            own.T.reshape(KT, 128, 512).transpose(1, 0, 2)
        ).astype(ml_dtypes.bfloat16)
        # h8[p, j, r, t] = chunk[t, (2j+r)*128+p] * 64
        h8prev = np.ascontiguousarray(
            (prev.T * FP8_SCALE).reshape(16, 2, 128, 512).transpose(2, 0, 1, 3)
        ).astype(fp8)
        h8own = np.ascontiguousarray(
            (own.T * FP8_SCALE).reshape(16, 2, 128, 512).transpose(2, 0, 1, 3)
        ).astype(fp8)
        costab, sintab, maskmul = _host_tables(positions[b], s0)
        in_maps.append({
            "htprev": hTprev,
            "htown": hTown,
            "h8prev": h8prev,
            "h8own": h8own,
            "wv": wv,
            "wk8": wk8,
            "wq8": wq8,
            "wodr": wo,
            "costab": np.ascontiguousarray(costab),
            "sintab": np.ascontiguousarray(sintab),
            "maskst": np.ascontiguousarray(maskmul),
            "filt": filt_arr,
            "rotm": np.ascontiguousarray(rotm),
        })

    kw = {}
    if TRACE:
        kw = dict(trace=True, trace_cores=[1], stitch_traces=False)
    res = run_bass_kernel_spmd(nc, in_maps, core_ids=list(range(NCORES)), **kw)
    _LAST_RESULTS = res

    out_full = np.empty((B, S, HID), dtype=np.float32)
    for c in range(NCORES):
        b, s0 = c // 4, (c % 4) * CHUNK
        out_full[b, s0:s0 + CHUNK] = res.results[c]["out"]
    return out_full


if __name__ == "__main__":
    rng = np.random.default_rng(0)
    F = (H + 2 * KV) * D
    ins = {
        "hidden_states": rng.standard_normal((B, S, HID)).astype(np.float32) * 0.02,
        "positions": np.broadcast_to(np.arange(S, dtype=np.int32), (B, S)).copy(),
        "w_pack": rng.standard_normal((HID, F)).astype(np.float32) * HID ** -0.5,
        "w_o": rng.standard_normal((H * D, HID)).astype(np.float32) * (H * D) ** -0.5,
        "conv_k": rng.standard_normal((2, KV)).astype(np.float32) * 0.5,
        "conv_v": rng.standard_normal((2, KV)).astype(np.float32) * 0.5,
    }
    out = kernel(**ins)
    print("kernel ran, out shape", out.shape, "finite:", np.isfinite(out).all())


# revision 20
# speedup vs baseline: 1.2152x; 1.2152x over previous
"""Trainium2 Bass kernel for nn_BaichuanAttention_4801773437527 (v3).

Sequence-sharded across 8 NeuronCores: core c handles 512 query rows
(batch c//4, seq block (c%4)*512). No collectives.

v3 changes vs v2:
 - Q and K projections in fp8-e4m3 with DoubleRow perf mode (2 packed
   contraction tiles per matmul): ~1.5-2x PE rate on 330us of matmuls.
   Inputs are scaled x64 on host (fp8 mid-range); the resulting x4096
   scale on q and k rides linearly through smoothing/rope and is divided
   out inside the softmax exp's scale argument -- zero extra ops.
   Numerically safe: q/k only feed scores, which are tiny (probs shift
   measured 5e-6 on CPU).
 - V projection stays bf16 (v feeds the output directly).
 - paired-accumulator kv matmuls: one LDWEIGHTS serves prev+own halves
 - o_proj weight pool opened early; first tile prefetched mid-attention
 - first weight tile DMA issued before the hidden-state DMAs
"""
import sys
sys.path.insert(0, '/opt/trn_rl_repo')
from contextlib import ExitStack
import numpy as np

B, S, HID = 2, 2048, 4096
H, KV, D = 32, 8, 128
WINDOW = 512
CHUNK = 512
NCORES = 8
ROPE_THETA = 10000.0
KT = HID // 128               # 32 contraction tiles (bf16); 16 fp8 pairs
SCALE = float(D) ** -0.5
FP8_SCALE = 64.0              # per-operand fp8 input scale
SCALE_QK = SCALE / (FP8_SCALE ** 4)   # q,k each carry x(64*64)

QRANGE = [(0, 128), (0, 256), (0, 384), (0, 512),
          (0, 512), (128, 384), (256, 256), (384, 128)]
KT_ORDER = [3, 0, 1, 2, 4, 5, 6, 7]

_PROGRAM = None
TRACE = False
_LAST_RESULTS = None


def _apply_patches():
    """This walrus build allows 1 sync wait per instruction (2 for
    EventSemaphore). Spill extra waits onto same-engine no-ops."""
    import concourse.mybir as mybir
    import concourse.tile as tile
    from concourse.vector_clock import ScopedClock

    if getattr(tile.TileContext, "_wait_patch_applied", False):
        return

    orig_lower = tile.TileContext._lower_ordered_insts
    counter = [0]

    def spill(ordered):
        for insts in ordered.values():
            new_insts = []
            for inst in insts:
                si = getattr(inst, "sync_info", None)
                if si is not None and type(inst).__name__.startswith("Inst"):
                    waits = list(si.on_wait)
                    cap = 2 if isinstance(inst, mybir.InstEventSemaphore) else 1
                    if len(waits) > cap:
                        for w in waits[cap:]:
                            counter[0] += 1
                            new_insts.append(mybir.InstNoOp(
                                name=f"wspill-{counter[0]}",
                                sync_info=mybir.SyncInfo(on_wait=[w], on_update=[]),
                                bass_nofuse=True,
                                engine=inst.engine,
                            ))
                        inst.sync_info = mybir.SyncInfo(
                            on_wait=waits[:cap], on_update=list(si.on_update))
                new_insts.append(inst)
            insts[:] = new_insts

    def patched_lower(self, ordered):
        spill(ordered)
        return orig_lower(self, ordered)

    def patched_drain_and_barrier(self, tick_clock, wait_clock):
        nc = self.nc
        collector = nc.sync.nop(nofuse=True)
        wait_clock.add_sem_waits(
            collector.ins, ScopedClock({None: tick_clock.global_clock}))
        si = collector.ins.sync_info
        waits = list(si.on_wait) if si is not None else []
        if len(waits) > 1:
            collector.ins.sync_info = mybir.SyncInfo(
                on_wait=[waits[0]], on_update=list(si.on_update))
            for w in waits[1:]:
                n = nc.sync.nop(nofuse=True)
                n.ins.sync_info = mybir.SyncInfo(on_wait=[w], on_update=[])
        nc.sync.drain()
        nc.all_engine_barrier()
        assert self.sems is not None
        popped = nc._tile_sem_poison_stack.pop()
        assert popped is self._sem_poison
        nc.clear_and_free_semaphores(list(self.sems.allocated().values()))
        nc.all_engine_barrier()

    tile.TileContext._lower_ordered_insts = patched_lower
    tile.TileContext._drain_and_barrier = patched_drain_and_barrier
    tile.TileContext._wait_patch_applied = True


def _build_program():
    import concourse.bass as bass
    import concourse.mybir as mybir
    import concourse.tile as tile
    from concourse.masks import make_identity

    _apply_patches()

    f32 = mybir.dt.float32
    bf16 = mybir.dt.bfloat16
    fp8 = mybir.dt.float8e4
    DR = mybir.MatmulPerfMode.DoubleRow
    MUL = mybir.AluOpType.mult
    ADD = mybir.AluOpType.add
    EXP = mybir.ActivationFunctionType.Exp

    nc = bass.Bass()
    # bf16 hidden (for V); fp8 hidden pre-paired [p, j, r, t] (for Q/K)
    hTprev = nc.dram_tensor("htprev", [128, KT, 512], bf16, kind="ExternalInput")
    hTown = nc.dram_tensor("htown", [128, KT, 512], bf16, kind="ExternalInput")
    h8prev = nc.dram_tensor("h8prev", [128, 16, 2, 512], fp8, kind="ExternalInput")
    h8own = nc.dram_tensor("h8own", [128, 16, 2, 512], fp8, kind="ExternalInput")
    wv = nc.dram_tensor("wv", [128, 8, KT, 128], bf16, kind="ExternalInput")
    wk8 = nc.dram_tensor("wk8", [128, 8, 16, 2, 128], fp8, kind="ExternalInput")
    wq8 = nc.dram_tensor("wq8", [128, 32, 16, 2, 128], fp8, kind="ExternalInput")
    wodr = nc.dram_tensor("wodr", [128, 8, KT, 512], bf16, kind="ExternalInput")
    costab = nc.dram_tensor("costab", [128, 1024], bf16, kind="ExternalInput")
    sintab = nc.dram_tensor("sintab", [128, 1024], bf16, kind="ExternalInput")
    maskst = nc.dram_tensor("maskst", [128, 8, 512], bf16, kind="ExternalInput")
    filt = nc.dram_tensor("filt", [128, 32], f32, kind="ExternalInput")
    rotm = nc.dram_tensor("rotm", [128, 128], bf16, kind="ExternalInput")
    out = nc.dram_tensor("out", [CHUNK, HID], f32, kind="ExternalOutput")
    out_r = out[:].rearrange("(st p) h -> st p h", p=128)   # [4,128,4096]

    with tile.TileContext(nc) as tc, ExitStack() as top:
        constp = top.enter_context(tc.tile_pool(name="const", bufs=1))
        ident_f = constp.tile([128, 128], f32, tag="identf")
        make_identity(nc, ident_f[:])
        ident = constp.tile([128, 128], bf16, tag="ident")
        nc.vector.tensor_copy(ident[:], ident_f[:])
        ones2d = constp.tile([128, 128], bf16, tag="ones2d")
        nc.gpsimd.memset(ones2d[:], 1.0)
        rot_sb = constp.tile([128, 128], bf16, tag="rot")
        nc.sync.dma_start(rot_sb[:], rotm[:])
        cos_sb = constp.tile([128, 1024], bf16, tag="cos")
        sin_sb = constp.tile([128, 1024], bf16, tag="sin")
        nc.sync.dma_start(cos_sb[:], costab[:])
        nc.sync.dma_start(sin_sb[:], sintab[:])
        filt_sb = constp.tile([128, 32], f32, tag="filt")
        nc.sync.dma_start(filt_sb[:], filt[:])

        es_at = ExitStack()
        atp = es_at.enter_context(tc.tile_pool(name="atp", bufs=1))
        attnT = atp.tile([128, H, 512], bf16, tag="attnT")

        es_kv = ExitStack()
        kvfix = es_kv.enter_context(tc.tile_pool(name="kvfix", bufs=1))
        kT_all = kvfix.tile([128, KV, 1024], bf16, tag="kTall")
        vT_all = kvfix.tile([128, KV * 8, 128], bf16, tag="vTall")

        es_h8o = ExitStack()
        h8op = es_h8o.enter_context(tc.tile_pool(name="h8op", bufs=1))
        h8own_sb = h8op.tile([128, 16, 2, 512], fp8, tag="h8own")
        es_hbo = ExitStack()
        hbop = es_hbo.enter_context(tc.tile_pool(name="htownp", bufs=1))
        htown_sb = hbop.tile([128, KT, 512], bf16, tag="htown")
        es_vw = ExitStack()
        vw = es_vw.enter_context(tc.tile_pool(name="vw", bufs=2))
        es_h8p = ExitStack()
        h8pp = es_h8p.enter_context(tc.tile_pool(name="h8pp", bufs=1))
        h8prev_sb = h8pp.tile([128, 16, 2, 512], fp8, tag="h8prev")

        # ---- phase Kk: K projection (fp8 DoubleRow) + smooth + rope ----
        vwts = []
        with tc.tile_pool(name="kw8", bufs=2) as kw8, \
             tc.tile_pool(name="kps", bufs=2, space="PSUM") as kps, \
             tc.tile_pool(name="krot", bufs=1, space="PSUM") as krot, \
             tc.tile_pool(name="smpk", bufs=2) as smpk:
            # DMA order: first K weight tile, then own/prev fp8 hidden in
            # chunks, remaining K weights, then next-phase prefetches
            kwts = []
            for i in range(8):
                wt = kw8.tile([128, 16, 2, 128], fp8, tag="kw")
                nc.sync.dma_start(wt[:], wk8[:, i])
                kwts.append(wt)
                if i == 0:
                    for cc in range(4):
                        nc.sync.dma_start(h8own_sb[:, 4 * cc:4 * cc + 4],
                                          h8own[:, 4 * cc:4 * cc + 4])
                    for cc in range(4):
                        nc.sync.dma_start(h8prev_sb[:, 4 * cc:4 * cc + 4],
                                          h8prev[:, 4 * cc:4 * cc + 4])
            # prefetch phase-V inputs while K computes
            for cc in range(4):
                nc.sync.dma_start(htown_sb[:, 8 * cc:8 * cc + 8, :],
                                  hTown[:, 8 * cc:8 * cc + 8, :])
            wt0 = vw.tile([128, KT, 128], bf16, tag="vw", name="vw0")
            nc.sync.dma_start(wt0[:], wv[:, 0])
            vwts.append(wt0)
            for i in range(8):
                wt = kwts[i]
                ps0 = kps.tile([128, 512], f32, tag="kps0")
                ps1 = kps.tile([128, 512], f32, tag="kps1")
                for j in range(16):
                    nc.tensor.matmul(ps1[:], wt[:, j], h8own_sb[:, j],
                                     start=(j == 0), stop=(j == 15),
                                     perf_mode=DR)
                    nc.tensor.matmul(ps0[:], wt[:, j], h8prev_sb[:, j],
                                     start=(j == 0), stop=(j == 15),
                                     perf_mode=DR)
                raw = smpk.tile([128, 1024], bf16, tag="kvraw")
                nc.scalar.copy(raw[:, 0:512], ps0[:])
                nc.scalar.copy(raw[:, 512:1024], ps1[:])
                f0 = filt_sb[:, i:i + 1]
                f1 = filt_sb[:, 8 + i:8 + i + 1]
                tmp = smpk.tile([128, 1024], bf16, tag="smtmp")
                nc.vector.tensor_scalar_mul(tmp[:], raw[:], f1)
                smo = smpk.tile([128, 1024], bf16, tag="smo")
                nc.vector.tensor_copy(smo[:, 0:1], tmp[:, 0:1])
                nc.vector.scalar_tensor_tensor(
                    smo[:, 1:1024], raw[:, 0:1023], f0, tmp[:, 1:1024], MUL, ADD)
                zps = krot.tile([128, 1024], f32, tag="zk")
                nc.tensor.matmul(zps[:, 0:512], rot_sb[:], smo[:, 0:512],
                                 start=True, stop=True)
                nc.tensor.matmul(zps[:, 512:1024], rot_sb[:], smo[:, 512:1024],
                                 start=True, stop=True)
                zsb = smpk.tile([128, 1024], bf16, tag="zsb")
                nc.scalar.copy(zsb[:], zps[:])
                t1 = smpk.tile([128, 1024], bf16, tag="kt1")
                nc.vector.tensor_tensor(t1[:], smo[:], cos_sb[:], MUL)
                t2 = smpk.tile([128, 1024], bf16, tag="kt2")
                nc.vector.tensor_tensor(t2[:], zsb[:], sin_sb[:], MUL)
                nc.vector.tensor_tensor(kT_all[:, i, :], t1[:], t2[:], ADD)
        es_h8p.close()

        # ---- phase V: V projection (bf16) + smooth + transpose ----
        es_hbp = ExitStack()
        hbpp = es_hbp.enter_context(tc.tile_pool(name="htprevp", bufs=1))
        htprev_sb = hbpp.tile([128, KT, 512], bf16, tag="htprev")
        with tc.tile_pool(name="vps", bufs=2, space="PSUM") as vps, \
             tc.tile_pool(name="vtps", bufs=2, space="PSUM") as vtps, \
             tc.tile_pool(name="smpv", bufs=2) as smpv:
            for cc in range(4):
                nc.sync.dma_start(
                    htprev_sb[:, 8 * cc:8 * cc + 8, :],
                    hTprev[:, 8 * cc:8 * cc + 8, :])
            for i in range(1, 8):
                wt = vw.tile([128, KT, 128], bf16, tag="vw")
                nc.sync.dma_start(wt[:], wv[:, i])
                vwts.append(wt)
            for i in range(8):
                wt = vwts[i]
                ps0 = vps.tile([128, 512], f32, tag="vps0")
                ps1 = vps.tile([128, 512], f32, tag="vps1")
                for kt in range(KT):
                    nc.tensor.matmul(ps1[:], wt[:, kt, :], htown_sb[:, kt, :],
                                     start=(kt == 0), stop=(kt == KT - 1))
                    nc.tensor.matmul(ps0[:], wt[:, kt, :], htprev_sb[:, kt, :],
                                     start=(kt == 0), stop=(kt == KT - 1))
                raw = smpv.tile([128, 1024], bf16, tag="vraw")
                nc.scalar.copy(raw[:, 0:512], ps0[:])
                nc.scalar.copy(raw[:, 512:1024], ps1[:])
                f0 = filt_sb[:, 16 + i:16 + i + 1]
                f1 = filt_sb[:, 24 + i:24 + i + 1]
                tmp = smpv.tile([128, 1024], bf16, tag="vtmp")
                nc.vector.tensor_scalar_mul(tmp[:], raw[:], f1)
                smo = smpv.tile([128, 1024], bf16, tag="vsmo")
                nc.vector.tensor_copy(smo[:, 0:1], tmp[:, 0:1])
                nc.vector.scalar_tensor_tensor(
                    smo[:, 1:1024], raw[:, 0:1023], f0, tmp[:, 1:1024], MUL, ADD)
                for tt in range(8):
                    pv = vtps.tile([128, 128], bf16, tag="vtp")
                    nc.tensor.transpose(
                        pv[:], smo[:, tt * 128:(tt + 1) * 128], ident[:])
                    nc.vector.tensor_copy(vT_all[:, i * 8 + tt, :], pv[:])
        es_hbp.close()
        es_vw.close()
        es_hbo.close()

        # ---- phase QA: per-head q projection (fp8 DR) + rope + attention ----
        es_msk = ExitStack()
        maskp = es_msk.enter_context(tc.tile_pool(name="maskp", bufs=1))
        mask_sb = maskp.tile([128, 8, 512], bf16, tag="mask")
        nc.sync.dma_start(mask_sb[:], maskst[:])
        es_wo0 = ExitStack()
        wop = es_wo0.enter_context(tc.tile_pool(name="wop", bufs=3))
        wodr_h = wodr[:].rearrange("p hc (kh ko) c -> p (hc kh) ko c", kh=2)
        owts = [None] * 3     # first 3 half-KT o_proj weight tiles
        with tc.tile_pool(name="qw", bufs=2) as qw, \
             tc.tile_pool(name="qps", bufs=2, space="PSUM") as qps, \
             tc.tile_pool(name="rps", bufs=1, space="PSUM") as rps, \
             tc.tile_pool(name="sps", bufs=2, space="PSUM") as sps, \
             tc.tile_pool(name="sums", bufs=1, space="PSUM") as sums, \
             tc.tile_pool(name="pvps", bufs=2, space="PSUM") as pvps, \
             tc.tile_pool(name="prp", bufs=2) as prp, \
             tc.tile_pool(name="qtmp", bufs=2) as qtmp:
            for h in range(H):
                g = h // (H // KV)
                wt = qw.tile([128, 16, 2, 128], fp8, tag="qwt")
                nc.sync.dma_start(wt[:], wq8[:, h])
                if h in (8, 16, 24):
                    oi = (h - 8) // 8
                    owts[oi] = wop.tile([128, 16, 512], bf16, tag="wo",
                                        name=f"wo{oi}")
                    nc.sync.dma_start(owts[oi][:], wodr_h[:, oi])
                ps = qps.tile([128, 512], f32, tag="qps")
                for j in range(16):
                    nc.tensor.matmul(ps[:], wt[:, j], h8own_sb[:, j],
                                     start=(j == 0), stop=(j == 15),
                                     perf_mode=DR)
                qraw = qtmp.tile([128, 512], bf16, tag="qraw")
                nc.scalar.copy(qraw[:], ps[:])
                zq = rps.tile([128, 512], f32, tag="zq")
                nc.tensor.matmul(zq[:], rot_sb[:], qraw[:], start=True, stop=True)
                zqs = qtmp.tile([128, 512], bf16, tag="zqs")
                nc.scalar.copy(zqs[:], zq[:])
                t1 = qtmp.tile([128, 512], bf16, tag="qt1")
                nc.vector.tensor_tensor(t1[:], qraw[:], cos_sb[:, 512:1024], MUL)
                t2 = qtmp.tile([128, 512], bf16, tag="qt2")
                nc.vector.tensor_tensor(t2[:], zqs[:], sin_sb[:, 512:1024], MUL)
                qf = qtmp.tile([128, 512], bf16, tag="qf")
                nc.vector.tensor_tensor(qf[:], t1[:], t2[:], ADD)

                probs = prp.tile([128, 8, 512], bf16, tag="probs")
                for kt in range(8):
                    qo, qn = QRANGE[kt]
                    sc = sps.tile([128, 512], f32, tag="sc")
                    nc.tensor.matmul(
                        sc[:, qo:qo + qn],
                        kT_all[:, g, kt * 128:(kt + 1) * 128],
                        qf[:, qo:qo + qn], start=True, stop=True)
                    nc.scalar.activation(
                        probs[:, kt, qo:qo + qn], sc[:, qo:qo + qn],
                        EXP, scale=SCALE_QK)
                    nc.vector.tensor_tensor(
                        probs[:, kt, qo:qo + qn], probs[:, kt, qo:qo + qn],
                        mask_sb[:, kt, qo:qo + qn], MUL)
                sm_ps = sums.tile([128, 512], f32, tag="sum")
                for j, kt in enumerate(KT_ORDER):
                    qo, qn = QRANGE[kt]
                    nc.tensor.matmul(
                        sm_ps[:, qo:qo + qn], ones2d[:],
                        probs[:, kt, qo:qo + qn],
                        start=(j == 0), stop=(j == 7))
                rec = qtmp.tile([128, 512], f32, tag="rec")
                nc.vector.reciprocal(rec[:], sm_ps[:])
                pv = pvps.tile([128, 512], f32, tag="pv")
                for j, kt in enumerate(KT_ORDER):
                    qo, qn = QRANGE[kt]
                    nc.tensor.matmul(
                        pv[:, qo:qo + qn], vT_all[:, g * 8 + kt, :],
                        probs[:, kt, qo:qo + qn],
                        start=(j == 0), stop=(j == 7))
                nc.vector.tensor_tensor(attnT[:, h, :], pv[:], rec[:], MUL)

        # ---- phase O: o_proj (half-KT weight tiles) ----
        with tc.tile_pool(name="ops", bufs=4, space="PSUM") as ops, \
             tc.tile_pool(name="oev", bufs=3) as oev:
            for hc in range(8):
                wts = []
                for kh in range(2):
                    t = 2 * hc + kh
                    if t < 3:
                        wts.append(owts[t])
                    else:
                        wt = wop.tile([128, 16, 512], bf16, tag="wo")
                        nc.sync.dma_start(wt[:], wodr_h[:, t])
                        wts.append(wt)
                for st in range(4):
                    ps = ops.tile([128, 512], f32, tag="ops")
                    for ft in range(KT):
                        nc.tensor.matmul(
                            ps[:], attnT[:, ft, st * 128:(st + 1) * 128],
                            wts[ft // 16][:, ft % 16, :],
                            start=(ft == 0), stop=(ft == KT - 1))
                    ev = oev.tile([128, 512], f32, tag="oev")
                    nc.scalar.copy(ev[:], ps[:])
                    nc.sync.dma_start(
                        out_r[st, :, hc * 512:(hc + 1) * 512], ev[:])
        es_wo0.close()
        es_msk.close()
        es_h8o.close()
        es_kv.close()
        es_at.close()
    return nc


def _rot_matrix():
    R = np.zeros((128, 128), np.float32)
    for d in range(64):
        R[d + 64, d] = -1.0
        R[d, d + 64] = 1.0
    return R


def _host_tables(positions_b, s0):
    """bf16 cos/sin rope tables [128,1024] and {0,1} bf16 mask [128,8,512]."""
    import ml_dtypes
    if s0 > 0:
        pos_prev = positions_b[s0 - 512:s0].astype(np.float64)
    else:
        pos_prev = np.zeros(512, np.float64)
    pos_own = positions_b[s0:s0 + 512].astype(np.float64)
    tpos = np.concatenate([pos_prev, pos_own])                   # [1024]
    inv = 1.0 / (ROPE_THETA ** (np.arange(64, dtype=np.float64) / 64.0))
    ang = inv[:, None] * tpos[None, :]                           # [64,1024]
    cos = np.cos(ang)
    sin = np.sin(ang)
    costab = np.concatenate([cos, cos], axis=0).astype(ml_dtypes.bfloat16)
    sintab = np.concatenate([sin, sin], axis=0).astype(ml_dtypes.bfloat16)

    t_idx = s0 - 512 + np.arange(1024)
    q_idx = s0 + np.arange(512)
    diff = q_idx[None, :] - t_idx[:, None]                       # [1024,512]
    valid = (diff >= 0) & (diff < WINDOW) & (t_idx[:, None] >= 0)
    maskmul = valid.astype(ml_dtypes.bfloat16)                   # [1024,512]
    maskmul = np.ascontiguousarray(
        maskmul.reshape(8, 128, 512).transpose(1, 0, 2))         # [128,8,512]
    return costab, sintab, maskmul


_WCACHE = {}


def _prep_weights(w_pack, w_o):
    import ml_dtypes
    fp8 = ml_dtypes.float8_e4m3
    key = (w_pack.__array_interface__['data'][0],
           w_o.__array_interface__['data'][0])
    if _WCACHE.get('key') == key:
        return _WCACHE['res']
    wp4 = w_pack.reshape(KT, 128, 48, 128)          # [ko, p, ft, c]
    # V tiles (orig f-tiles 40..47), bf16: wv[p, i, ko, c]
    wv = np.ascontiguousarray(
        wp4[:, :, 40:48, :].transpose(1, 2, 0, 3)).astype(ml_dtypes.bfloat16)
    # K tiles (32..39) fp8 x64, paired ko: wk8[p, i, j, r, c]
    wk = (wp4[:, :, 32:40, :] * FP8_SCALE).astype(fp8)   # [ko, p, i, c]
    wk8 = np.ascontiguousarray(
        wk.reshape(16, 2, 128, 8, 128).transpose(2, 3, 0, 1, 4))
    # Q tiles (0..31) fp8 x64
    wq = (wp4[:, :, 0:32, :] * FP8_SCALE).astype(fp8)
    wq8 = np.ascontiguousarray(
        wq.reshape(16, 2, 128, 32, 128).transpose(2, 3, 0, 1, 4))
    # wodr[p, hc, ko, c] = w_o[ko*128+p, hc*512+c]
    wo4 = w_o.reshape(KT, 128, 8, 512)
    wo = np.ascontiguousarray(
        wo4.transpose(1, 2, 0, 3)).astype(ml_dtypes.bfloat16)
    res = (wv, wk8, wq8, wo)
    _WCACHE.update(key=key, res=res)
    return res


def kernel(**inputs) -> np.ndarray:
    global _PROGRAM, _LAST_RESULTS
    from concourse.bass_utils import run_bass_kernel_spmd
    import ml_dtypes
    fp8 = ml_dtypes.float8_e4m3

    hidden = np.ascontiguousarray(inputs["hidden_states"], dtype=np.float32)
    positions = np.asarray(inputs["positions"], dtype=np.int32)
    w_pack = np.ascontiguousarray(inputs["w_pack"], dtype=np.float32)
    w_o = np.ascontiguousarray(inputs["w_o"], dtype=np.float32)
    conv_k = np.asarray(inputs["conv_k"], dtype=np.float32)
    conv_v = np.asarray(inputs["conv_v"], dtype=np.float32)

    if _PROGRAM is None:
        _PROGRAM = _build_program()
    nc = _PROGRAM

    wv, wk8, wq8, wo = _prep_weights(w_pack, w_o)
    filt_arr = np.concatenate(
        [conv_k[0], conv_k[1], conv_v[0], conv_v[1]]).reshape(1, 32)
    filt_arr = np.ascontiguousarray(
        np.tile(filt_arr, (128, 1)), dtype=np.float32)
    rotm = _rot_matrix().astype(ml_dtypes.bfloat16)

    in_maps = []
    for c in range(NCORES):
        b, s0 = c // 4, (c % 4) * CHUNK
        own = hidden[b, s0:s0 + CHUNK]
        prev = hidden[b, s0 - CHUNK:s0] if s0 > 0 else np.zeros_like(own)
        # hT[p, kt, t] = chunk[t, kt*128+p]
        hTprev = np.ascontiguousarray(
            prev.T.reshape(KT, 128, 512).transpose(1, 0, 2)
        ).astype(ml_dtypes.bfloat16)
        hTown = np.ascontiguousarray(
            own.T.reshape(KT, 128, 512).transpose(1, 0, 2)
        ).astype(ml_dtypes.bfloat16)
        # h8[p, j, r, t] = chunk[t, (2j+r)*128+p] * 64
        h8prev = np.ascontiguousarray(
            (prev.T * FP8_SCALE).reshape(16, 2, 128, 512).transpose(2, 0, 1, 3)
        ).astype(fp8)
        h8own = np.ascontiguousarray(
            (own.T * FP8_SCALE).reshape(16, 2, 128, 512).transpose(2, 0, 1, 3)
        ).astype(fp8)
        costab, sintab, maskmul = _host_tables(positions[b], s0)
        in_maps.append({
            "htprev": hTprev,
            "htown": hTown,
            "h8prev": h8prev,
            "h8own": h8own,
            "wv": wv,
            "wk8": wk8,
            "wq8": wq8,
            "wodr": wo,
            "costab": np.ascontiguousarray(costab),
            "sintab": np.ascontiguousarray(sintab),
            "maskst": np.ascontiguousarray(maskmul),
            "filt": filt_arr,
            "rotm": np.ascontiguousarray(rotm),
        })

    kw = {}
    if TRACE:
        kw = dict(trace=True, trace_cores=[1], stitch_traces=False)
    res = run_bass_kernel_spmd(nc, in_maps, core_ids=list(range(NCORES)), **kw)
    _LAST_RESULTS = res

    out_full = np.empty((B, S, HID), dtype=np.float32)
    for c in range(NCORES):
        b, s0 = c // 4, (c % 4) * CHUNK
        out_full[b, s0:s0 + CHUNK] = res.results[c]["out"]
    return out_full


if __name__ == "__main__":
    rng = np.random.default_rng(0)
    F = (H + 2 * KV) * D
    ins = {
        "hidden_states": rng.standard_normal((B, S, HID)).astype(np.float32) * 0.02,
        "positions": np.broadcast_to(np.arange(S, dtype=np.int32), (B, S)).copy(),
        "w_pack": rng.standard_normal((HID, F)).astype(np.float32) * HID ** -0.5,
        "w_o": rng.standard_normal((H * D, HID)).astype(np.float32) * (H * D) ** -0.5,
        "conv_k": rng.standard_normal((2, KV)).astype(np.float32) * 0.5,
        "conv_v": rng.standard_normal((2, KV)).astype(np.float32) * 0.5,
    }
    out = kernel(**ins)
    print("kernel ran, out shape", out.shape, "finite:", np.isfinite(out).all())
